# revision 28
# baseline (speedup 1.0000x reference)
"""CrossAttentionFusion Trainium2 kernel — fp8 DoubleRow edition.

Full inputs -> shard (batch x query-half) over 8 NeuronCores -> full output.

Per core (batch b = core//2, query half = core%2): NH=2048 queries n,
N=4096 keys m, C=256 channels.

Host precompute (exact f32, then fp8e4 quantization):
  Q'[c,n] = (q_w^T k_w)^T x1 + k_w^T q_b        (logits rhs)
  Y[o,m]  = G[o] * (p_w v_w x2)[o,m]            (fusion rhs; G = BN scale)
  gate[n] = sigmoid(gate_w [x1;x2] + gate_b)    (per-query scalar)
  Bc[o]   = beta + (p_b + p_w v_b - mean) * G   (post bias row)
Device per 512-query block j:
  L[m,n]  = x2^T Q'      fp8 DoubleRow matmuls (contraction c=256/instr)
  E       = exp(L/16 - 2.5) on ACT, fp8e4 out (offset cancels in Z/S)
  P[n,o]  = sum_m E[m,n] Y[o,m]  fp8 DR, out in [query, channel] layout;
            Y carries a ones column so P[:,256] = S = softmax denominator
  out^T   = x1^T + gate * relu(P[:, :256]/S + Bc)   (DVE recip/STT,
            GpSimd relu*gate, DVE residual add; all per-partition scalars)
  fusion(j-1) instrs interleave into logits(j) slots on the PE; exp is the
  pacing engine (~55us floor: 8.4M elements at 128 lanes / 1.2 GHz).
"""
from contextlib import ExitStack

import numpy as np
import ml_dtypes

import concourse.bass as bass
import concourse.mybir as mybir
import concourse.tile as tile
from concourse import bacc
from concourse.bass_utils import run_bass_kernel_spmd

F32 = mybir.dt.float32
BF16 = mybir.dt.bfloat16
FP8 = mybir.dt.float8e4
AF = mybir.ActivationFunctionType
OP = mybir.AluOpType
DR = mybir.MatmulPerfMode.DoubleRow
NP8 = ml_dtypes.float8_e4m3

B, C, H, W = 4, 256, 64, 64
N = H * W            # 4096 keys per batch
NCORES = 8
NH = N // 2          # 2048 queries per core
NBLK = 512           # query block
NBLOCKS = NH // NBLK
MT = N // 128        # 32 key tiles
MT2 = MT // 2        # 16 DoubleRow key-pair steps
YW = 272             # yt row: 256 channels + ones col + pad to %16
NT = NBLK // 128     # 4 query tiles per block
EPS = 1e-5
SCALE = float(C) ** -0.5
EOFF = 2.5           # exp offset; cancels in Z/S


def build():
    nc = bacc.Bacc("TRN2", target_bir_lowering=False, debug=False,
                   num_devices=NCORES)
    q8_d = nc.dram_tensor("q8", [128, 2 * NH], FP8, kind="ExternalInput")
    x2_d = nc.dram_tensor("x2dr", [128, 2 * N], FP8, kind="ExternalInput")
    yt_d = nc.dram_tensor("yt", [128, MT * YW], FP8, kind="ExternalInput")
    x1_d = nc.dram_tensor("x1t", [NH, C], BF16, kind="ExternalInput")
    gc_d = nc.dram_tensor("gatec", [128, NBLOCKS * NT], F32,
                          kind="ExternalInput")
    bc_d = nc.dram_tensor("bct", [128, C], F32, kind="ExternalInput")
    out_d = nc.dram_tensor("out", [NH, C], BF16, kind="ExternalOutput")

    with tile.TileContext(nc) as tc, ExitStack() as ctx:
        pers = ctx.enter_context(tc.tile_pool(name="pers", bufs=1))
        epool = ctx.enter_context(tc.tile_pool(name="epool", bufs=2))
        work = ctx.enter_context(tc.tile_pool(name="work", bufs=2))
        outs = ctx.enter_context(tc.tile_pool(name="outs", bufs=3))
        psL = ctx.enter_context(tc.tile_pool(name="psL", bufs=2, space="PSUM"))
        psF = ctx.enter_context(tc.tile_pool(name="psF", bufs=4, space="PSUM"))

        q8 = pers.tile([128, 2, NH], FP8, tag="q8", name="q8")
        # chunked tiles: dependency tracking is whole-tile, so chunk tiles
        # let block-0 compute start as soon as its chunk lands
        x2c = [pers.tile([128, 2, 1024], FP8, tag=f"x2c{c}", name=f"x2c{c}")
               for c in range(4)]
        ytc = [pers.tile([128, 4, YW], FP8, tag=f"ytc{i}", name=f"ytc{i}")
               for i in range(8)]
        x1p = pers.tile([128, NBLOCKS * NT, C], BF16, tag="x1p", name="x1p")
        bct = pers.tile([128, C], F32, tag="bct", name="bct")
        gc = pers.tile([128, NBLOCKS * NT], F32, tag="gc", name="gc")
        nbias = pers.tile([128, 1], F32, tag="nbias", name="nbias")

        def _x2(c):
            c0 = c * 1024
            nc.sync.dma_start(x2c[c][:, 0, :], x2_d[:, c0:c0 + 1024])
            nc.gpsimd.dma_start(x2c[c][:, 1, :],
                                x2_d[:, N + c0:N + c0 + 1024])

        def _yt(t0):
            eng = nc.sync if (t0 // 4) % 2 == 0 else nc.gpsimd
            eng.dma_start(
                ytc[t0 // 4][:],
                yt_d[:, t0 * YW:(t0 + 4) * YW].rearrange(
                    "p (t y) -> p t y", y=YW))

        with nc.named_scope("pre"):
            nc.vector.memset(nbias[:], -EOFF)
            # PE warmup: ramp the clock out of the low p-state on scratch
            # data while the first DMAs land.
            wl = pers.tile([128, 2, 128], FP8, tag="wl", name="wl")
            wr = pers.tile([128, 2, 257], FP8, tag="wr", name="wr")
            nc.vector.memset(wl[:], 0.0)
            nc.vector.memset(wr[:], 0.0)
            for w in range(6):
                wp = psF.tile([128, 257], F32, tag="fuse", name="fuse")
                nc.tensor.matmul(wp[:], wl[:], wr[:], start=True, stop=True,
                                 perf_mode=DR)
            nc.sync.dma_start(q8[:, 0, :], q8_d[:, 0:NH])
            nc.gpsimd.dma_start(q8[:, 1, :], q8_d[:, NH:2 * NH])
            _x2(0)

        def dma_feed():
            # Deferred DMA issues, one step per slot, just ahead of need.
            # Consumers wait on all prior issues of a queue, so issuing late
            # (but before the consuming instr is emitted) is what overlaps
            # transfers with block-0 compute.
            _x2(1); _yt(0); _yt(4)
            yield
            _x2(2)
            yield
            _yt(8); _yt(12)
            yield
            _x2(3)
            yield
            _yt(16); _yt(20)
            yield
            _yt(24); _yt(28)
            yield
            nc.sync.dma_start(bct[:], bc_d[:])
            nc.sync.dma_start(gc[:], gc_d[:])
            yield
            for t in range(NBLOCKS * NT):
                eng = nc.gpsimd if t % 2 == 0 else nc.sync
                eng.dma_start(x1p[:, t, :], x1_d[t * 128:(t + 1) * 128, :])
                if t % 2 == 1:
                    yield

        feed = dma_feed()

        def emit_fusion(e8, nt, k, fuse, fcnt):
            if fcnt[nt] == 0:
                fuse[nt] = psF.tile([128, 257], F32, tag="fuse", name="fuse")
            sub = (2 * k) % 4
            ex = e8[k % 2]
            kk = (k // 2) * 2
            nc.tensor.matmul(
                fuse[nt][:, 0:257],
                ex[:, kk:kk + 2, nt * 128:(nt + 1) * 128],
                ytc[k // 2][:, sub:sub + 2, 0:257],
                start=(fcnt[nt] == 0), stop=(fcnt[nt] == MT2 - 1),
                perf_mode=DR)
            fcnt[nt] += 1

        def emit_posts(j, fuse):
            # copy PSUM out first (releases the fuse banks for the next
            # block's accumulators), then the per-nt postludes
            cps = []
            with nc.named_scope(f"post{j}"):
                for nt in range(NT):
                    cp = work.tile([128, 257], F32, tag=f"cp{nt}",
                                   name=f"cp{nt}")
                    nc.vector.tensor_copy(cp[:], fuse[nt][:])
                    cps.append(cp)
                for nt in range(NT):
                    t_idx = j * NT + nt
                    cp = cps[nt]
                    invs = work.tile([128, 1], F32, tag="invs", name="invs")
                    nc.vector.reciprocal_approx_fast(invs[:], cp[:, 256:257])
                    tt = work.tile([128, C], F32, tag="tt", name="tt")
                    nc.vector.scalar_tensor_tensor(
                        tt[:], cp[:, 0:256], invs[:], bct[:],
                        op0=OP.mult, op1=OP.add)
                    rg = work.tile([128, C], F32, tag="rg", name="rg")
                    nc.vector.tensor_scalar_max(rg[:], tt[:], 0.0)
                    ot = outs.tile([128, C], BF16, tag="ot", name="ot")
                    nc.vector.scalar_tensor_tensor(
                        ot[:], rg[:], gc[:, t_idx:t_idx + 1],
                        x1p[:, t_idx, :], op0=OP.mult, op1=OP.add)
                    nc.sync.dma_start(
                        out_d[t_idx * 128:(t_idx + 1) * 128, :], ot[:])

        # Hybrid exp: most slots on ACT; DVE_KS slots use the Schraudolph
        # bit-trick (y*2^23/ln2 + magic as int32, bitcast = approx exp).
        DVE_KS = (8, 11, 14)
        A_S = (8388608.0 / float(np.log(2.0))) * SCALE
        B_S = 1065353216.0 - 366393.0 - EOFF * (8388608.0 / float(np.log(2.0)))

        def emit_exp(e8, mt2, lp, dve_ks):
            ex = e8[mt2 % 2]
            kk = (mt2 // 2) * 2
            dst = ex[:, kk:kk + 2, :]
            if mt2 in dve_ks:
                it = work.tile([128, 2, NBLK], mybir.dt.int32, tag="i32",
                               name="i32")
                nc.vector.tensor_scalar(it[:], lp[:], A_S, B_S,
                                        op0=OP.mult, op1=OP.add)
                nc.vector.tensor_copy(dst, it[:].bitcast(F32))
            else:
                nc.scalar.activation(dst, lp[:], AF.Exp, scale=SCALE,
                                     bias=nbias[:])

        # FIFO fusion scheduler: each exp(k) enqueues its 4 fusion instrs at
        # slot k+1 (ACT) or k+3 (DVE, E8 lands later); each slot pops at most
        # CAP so the ACT-critical logits never queue behind a fusion bunch.
        CAP = 5
        fq = []
        avail_at = {}

        def pump(budget):
            while fq and budget > 0:
                e8x, nt, k, fusex, fcntx, jx = fq.pop(0)
                emit_fusion(e8x, nt, k, fusex, fcntx)
                budget -= 1
                if fcntx[nt] == MT2 and all(c == MT2 for c in fcntx):
                    emit_posts(jx, fusex)

        S = 0
        for j in range(NBLOCKS):
            ns = slice(j * NBLK, (j + 1) * NBLK)
            with nc.named_scope(f"blk{j}"):
                e8 = [epool.tile([128, MT2, NBLK], FP8, tag=f"E8{p}",
                                 name=f"E8{p}") for p in range(2)]
                fuse = {}
                fcnt = [0] * NT
                dve_ks = (5,) + DVE_KS if j == 0 else DVE_KS
                for mt2 in range(MT2):
                    lp = psL.tile([128, 2, NBLK], F32, tag="L", name="L")
                    for sub in range(2):
                        mt = 2 * mt2 + sub
                        cc = (mt % 8) * 128
                        nc.tensor.matmul(
                            lp[:, sub, :],
                            x2c[mt // 8][:, :, cc:cc + 128],
                            q8[:, :, ns], start=True, stop=True, perf_mode=DR)
                    for item in avail_at.pop(S, ()):
                        fq.append(item)
                    pump(CAP)
                    emit_exp(e8, mt2, lp, dve_ks)
                    lag = 3 if mt2 in dve_ks else 1
                    avail_at.setdefault(S + lag, []).extend(
                        (e8, nt, mt2, fuse, fcnt, j) for nt in range(NT))
                    next(feed, None)
                    S += 1
        with nc.named_scope("tail"):
            while fq or avail_at:
                for item in avail_at.pop(S, ()):
                    fq.append(item)
                pump(CAP)
                S += 1
    nc.compile()
    return nc


_NC = None


def _get_nc():
    global _NC
    if _NC is None:
        _NC = build()
    return _NC


def kernel(**inputs):
    x1 = np.asarray(inputs["x1"], np.float32).reshape(B, C, N)
    x2 = np.asarray(inputs["x2"], np.float32).reshape(B, C, N)
    q_w = np.asarray(inputs["q_w"], np.float32)
    k_w = np.asarray(inputs["k_w"], np.float32)
    v_w = np.asarray(inputs["v_w"], np.float32)
    p_w = np.asarray(inputs["proj_w"], np.float32)
    q_b = np.asarray(inputs["q_b"], np.float32)
    v_b = np.asarray(inputs["v_b"], np.float32)
    p_b = np.asarray(inputs["proj_b"], np.float32)
    gamma = np.asarray(inputs["bn_gamma"], np.float32)
    beta = np.asarray(inputs["bn_beta"], np.float32)
    mean = np.asarray(inputs["bn_mean"], np.float32)
    var = np.asarray(inputs["bn_var"], np.float32)
    gate_w = np.asarray(inputs["gate_w"], np.float32)
    gate_b = np.asarray(inputs["gate_b"], np.float32)

    wqk = q_w.T @ k_w                      # [C,C]
    A = gamma[:, None] / np.sqrt(var + EPS)[:, None] * (p_w @ v_w)  # G*(pw vw)
    G = gamma / np.sqrt(var + EPS)
    Bc = (beta + (p_b + p_w @ v_b - mean) * G).astype(np.float32)
    qpb = (k_w.T @ q_b).astype(np.float32)
    bct = np.ascontiguousarray(np.broadcast_to(Bc, (128, C)))

    in_maps = []
    for b in range(B):
        Qp = (wqk.T @ x1[b] + qpb[:, None]).astype(NP8)      # [C, N]
        Y8 = (A @ x2[b]).astype(NP8)                          # [C, N]
        glog = gate_w[0, :C] @ x1[b] + gate_w[0, C:] @ x2[b] + gate_b[0]
        gate = (1.0 / (1.0 + np.exp(-glog))).astype(np.float32)  # [N]
        x28 = x2[b].astype(NP8)
        # x2dr/q8 layouts: [p, h, m] = arr[h*128+p, m]
        x2dr = np.ascontiguousarray(
            x28.reshape(2, 128, N).transpose(1, 0, 2).reshape(128, 2 * N))
        yt = np.zeros((128, MT, YW), NP8)
        yt[:, :, :C] = np.ascontiguousarray(
            Y8.reshape(C, MT, 128).transpose(2, 1, 0))
        yt[:, :, C] = np.float32(1.0)
        yt = np.ascontiguousarray(yt.reshape(128, MT * YW))
        for half in range(2):
            hq = slice(half * NH, (half + 1) * NH)
            q8 = np.ascontiguousarray(
                Qp[:, hq].reshape(2, 128, NH).transpose(1, 0, 2)
                .reshape(128, 2 * NH))
            x1t = np.ascontiguousarray(
                x1[b][:, hq].T.astype(ml_dtypes.bfloat16))    # [NH, C]
            gc = np.ascontiguousarray(
                gate[hq].reshape(NBLOCKS * NT, 128).T.astype(np.float32))
            in_maps.append({
                "q8": q8, "x2dr": x2dr, "yt": yt, "x1t": x1t,
                "gatec": gc, "bct": bct,
            })

    nc = _get_nc()
    res = run_bass_kernel_spmd(nc, in_maps, core_ids=list(range(NCORES)))
    out = np.empty((B, C, N), np.float32)
    for core in range(NCORES):
        b, half = divmod(core, 2)
        out[b, :, half * NH:(half + 1) * NH] = \
            res.results[core]["out"].astype(np.float32).T
    return out.reshape(B, C, H, W)


# revision 29
# speedup vs baseline: 1.0802x; 1.0802x over previous
"""CrossAttentionFusion Trainium2 kernel — fp8 DoubleRow edition.

Full inputs -> shard (batch x query-half) over 8 NeuronCores -> full output.

Per core (batch b = core//2, query half = core%2): NH=2048 queries n,
N=4096 keys m, C=256 channels.

Host precompute (exact f32, then fp8e4 quantization):
  Q'[c,n] = (q_w^T k_w)^T x1 + k_w^T q_b        (logits rhs)
  Y[o,m]  = G[o] * (p_w v_w x2)[o,m]            (fusion rhs; G = BN scale)
  gate[n] = sigmoid(gate_w [x1;x2] + gate_b)    (per-query scalar)
  Bc[o]   = beta + (p_b + p_w v_b - mean) * G   (post bias row)
Device per 512-query block j:
  L[m,n]  = x2^T Q'      fp8 DoubleRow matmuls (contraction c=256/instr)
  E       = exp(L/16 - 2.5) on ACT, fp8e4 out (offset cancels in Z/S)
  P[n,o]  = sum_m E[m,n] Y[o,m]  fp8 DR, out in [query, channel] layout;
            Y carries a ones column so P[:,256] = S = softmax denominator
  out^T   = x1^T + gate * relu(P[:, :256]/S + Bc)   (DVE recip/STT,
            GpSimd relu*gate, DVE residual add; all per-partition scalars)
  fusion(j-1) instrs interleave into logits(j) slots on the PE; exp is the
  pacing engine (~55us floor: 8.4M elements at 128 lanes / 1.2 GHz).
"""
from contextlib import ExitStack

import numpy as np
import ml_dtypes

import concourse.bass as bass
import concourse.mybir as mybir
import concourse.tile as tile
from concourse import bacc
from concourse.bass_utils import run_bass_kernel_spmd

F32 = mybir.dt.float32
BF16 = mybir.dt.bfloat16
FP8 = mybir.dt.float8e4
AF = mybir.ActivationFunctionType
OP = mybir.AluOpType
DR = mybir.MatmulPerfMode.DoubleRow
NP8 = ml_dtypes.float8_e4m3

B, C, H, W = 4, 256, 64, 64
N = H * W            # 4096 keys per batch
NCORES = 8
NH = N // 2          # 2048 queries per core
NBLK = 512           # query block
NBLOCKS = NH // NBLK
MT = N // 128        # 32 key tiles
MT2 = MT // 2        # 16 DoubleRow key-pair steps
YW = 272             # yt row: 256 channels + ones col + pad to %16
NT = NBLK // 128     # 4 query tiles per block
EPS = 1e-5
SCALE = float(C) ** -0.5
EOFF = 2.5           # exp offset; cancels in Z/S


def build():
    nc = bacc.Bacc("TRN2", target_bir_lowering=False, debug=False,
                   num_devices=NCORES)
    q8_d = nc.dram_tensor("q8", [128, 2 * NH], FP8, kind="ExternalInput")
    x2_d = nc.dram_tensor("x2dr", [128, 2 * N], FP8, kind="ExternalInput")
    yt_d = nc.dram_tensor("yt", [128, MT * YW], FP8, kind="ExternalInput")
    x1_d = nc.dram_tensor("x1t", [NH, C], BF16, kind="ExternalInput")
    gc_d = nc.dram_tensor("gatec", [128, NBLOCKS * NT], F32,
                          kind="ExternalInput")
    bc_d = nc.dram_tensor("bct", [128, C], F32, kind="ExternalInput")
    out_d = nc.dram_tensor("out", [NH, C], BF16, kind="ExternalOutput")

    with tile.TileContext(nc) as tc, ExitStack() as ctx:
        pers = ctx.enter_context(tc.tile_pool(name="pers", bufs=1))
        epool = ctx.enter_context(tc.tile_pool(name="epool", bufs=2))
        work = ctx.enter_context(tc.tile_pool(name="work", bufs=2))
        outs = ctx.enter_context(tc.tile_pool(name="outs", bufs=3))
        psL = ctx.enter_context(tc.tile_pool(name="psL", bufs=2, space="PSUM"))
        psF = ctx.enter_context(tc.tile_pool(name="psF", bufs=4, space="PSUM"))

        q8 = pers.tile([128, 2, NH], FP8, tag="q8", name="q8")
        # chunked tiles: dependency tracking is whole-tile, so chunk tiles
        # let block-0 compute start as soon as its chunk lands
        x2c = [pers.tile([128, 2, 1024], FP8, tag=f"x2c{c}", name=f"x2c{c}")
               for c in range(4)]
        ytc = [pers.tile([128, 4, YW], FP8, tag=f"ytc{i}", name=f"ytc{i}")
               for i in range(8)]
        x1p = pers.tile([128, NBLOCKS * NT, C], BF16, tag="x1p", name="x1p")
        bct = pers.tile([128, C], F32, tag="bct", name="bct")
        gc = pers.tile([128, NBLOCKS * NT], F32, tag="gc", name="gc")
        nbias = pers.tile([128, 1], F32, tag="nbias", name="nbias")

        def _x2(c):
            c0 = c * 1024
            nc.sync.dma_start(x2c[c][:, 0, :], x2_d[:, c0:c0 + 1024])
            nc.gpsimd.dma_start(x2c[c][:, 1, :],
                                x2_d[:, N + c0:N + c0 + 1024])

        def _yt(t0):
            eng = nc.sync if (t0 // 4) % 2 == 0 else nc.gpsimd
            eng.dma_start(
                ytc[t0 // 4][:],
                yt_d[:, t0 * YW:(t0 + 4) * YW].rearrange(
                    "p (t y) -> p t y", y=YW))

        with nc.named_scope("pre"):
            nc.vector.memset(nbias[:], -EOFF)
            # PE warmup: ramp the clock out of the low p-state on scratch
            # data while the first DMAs land.
            wl = pers.tile([128, 2, 128], FP8, tag="wl", name="wl")
            wr = pers.tile([128, 2, 257], FP8, tag="wr", name="wr")
            nc.vector.memset(wl[:], 0.0)
            nc.vector.memset(wr[:], 0.0)
            for w in range(6):
                wp = psF.tile([128, 257], F32, tag="fuse", name="fuse")
                nc.tensor.matmul(wp[:], wl[:], wr[:], start=True, stop=True,
                                 perf_mode=DR)
            nc.sync.dma_start(q8[:, 0, :], q8_d[:, 0:NH])
            nc.gpsimd.dma_start(q8[:, 1, :], q8_d[:, NH:2 * NH])
            _x2(0)

        def dma_feed():
            # Deferred DMA issues, one step per slot, just ahead of need.
            # Consumers wait on all prior issues of a queue, so issuing late
            # (but before the consuming instr is emitted) is what overlaps
            # transfers with block-0 compute.
            _x2(1); _yt(0); _yt(4)
            yield
            _x2(2)
            yield
            _yt(8); _yt(12)
            yield
            _x2(3)
            yield
            _yt(16); _yt(20)
            yield
            _yt(24); _yt(28)
            yield
            nc.sync.dma_start(bct[:], bc_d[:])
            nc.sync.dma_start(gc[:], gc_d[:])
            yield
            for t in range(NBLOCKS * NT):
                eng = nc.gpsimd if t % 2 == 0 else nc.sync
                eng.dma_start(x1p[:, t, :], x1_d[t * 128:(t + 1) * 128, :])
                if t % 2 == 1:
                    yield

        feed = dma_feed()

        def emit_fusion(e8, nt, k, fuse, fcnt):
            if fcnt[nt] == 0:
                fuse[nt] = psF.tile([128, 257], F32, tag="fuse", name="fuse")
            sub = (2 * k) % 4
            ex = e8[k % 2]
            kk = (k // 2) * 2
            nc.tensor.matmul(
                fuse[nt][:, 0:257],
                ex[:, kk:kk + 2, nt * 128:(nt + 1) * 128],
                ytc[k // 2][:, sub:sub + 2, 0:257],
                start=(fcnt[nt] == 0), stop=(fcnt[nt] == MT2 - 1),
                perf_mode=DR)
            fcnt[nt] += 1

        def emit_posts(j, fuse):
            # copy PSUM out first (releases the fuse banks for the next
            # block's accumulators), then the per-nt postludes
            cps = []
            with nc.named_scope(f"post{j}"):
                for nt in range(NT):
                    cp = work.tile([128, 257], F32, tag=f"cp{nt}",
                                   name=f"cp{nt}")
                    nc.vector.tensor_copy(cp[:], fuse[nt][:])
                    cps.append(cp)
                for nt in range(NT):
                    t_idx = j * NT + nt
                    cp = cps[nt]
                    invs = work.tile([128, 1], F32, tag="invs", name="invs")
                    nc.vector.reciprocal_approx_fast(invs[:], cp[:, 256:257])
                    tt = work.tile([128, C], F32, tag="tt", name="tt")
                    nc.vector.scalar_tensor_tensor(
                        tt[:], cp[:, 0:256], invs[:], bct[:],
                        op0=OP.mult, op1=OP.add)
                    rg = work.tile([128, C], F32, tag="rg", name="rg")
                    nc.vector.tensor_scalar_max(rg[:], tt[:], 0.0)
                    ot = outs.tile([128, C], BF16, tag="ot", name="ot")
                    nc.vector.scalar_tensor_tensor(
                        ot[:], rg[:], gc[:, t_idx:t_idx + 1],
                        x1p[:, t_idx, :], op0=OP.mult, op1=OP.add)
                    nc.sync.dma_start(
                        out_d[t_idx * 128:(t_idx + 1) * 128, :], ot[:])

        # Hybrid exp: most slots on ACT; DVE_KS slots use the Schraudolph
        # bit-trick (y*2^23/ln2 + magic as int32, bitcast = approx exp).
        DVE_KS = (8, 11, 14)
        A_S = (8388608.0 / float(np.log(2.0))) * SCALE
        B_S = 1065353216.0 - 366393.0 - EOFF * (8388608.0 / float(np.log(2.0)))

        def emit_exp(e8, mt2, lp, dve_ks):
            ex = e8[mt2 % 2]
            kk = (mt2 // 2) * 2
            dst = ex[:, kk:kk + 2, :]
            if mt2 in dve_ks:
                it = work.tile([128, 2, NBLK], mybir.dt.int32, tag="i32",
                               name="i32")
                nc.vector.tensor_scalar(it[:], lp[:], A_S, B_S,
                                        op0=OP.mult, op1=OP.add)
                nc.vector.tensor_copy(dst, it[:].bitcast(F32))
            else:
                nc.scalar.activation(dst, lp[:], AF.Exp, scale=SCALE,
                                     bias=nbias[:])

        # FIFO fusion scheduler: each exp(k) enqueues its 4 fusion instrs at
        # slot k+1 (ACT) or k+3 (DVE, E8 lands later); each slot pops at most
        # CAP so the ACT-critical logits never queue behind a fusion bunch.
        CAP = 5
        fq = []
        avail_at = {}

        def pump(budget):
            while fq and budget > 0:
                e8x, nt, k, fusex, fcntx, jx = fq.pop(0)
                emit_fusion(e8x, nt, k, fusex, fcntx)
                budget -= 1
                if fcntx[nt] == MT2 and all(c == MT2 for c in fcntx):
                    emit_posts(jx, fusex)

        S = 0
        for j in range(NBLOCKS):
            ns = slice(j * NBLK, (j + 1) * NBLK)
            with nc.named_scope(f"blk{j}"):
                e8 = [epool.tile([128, MT2, NBLK], FP8, tag=f"E8{p}",
                                 name=f"E8{p}") for p in range(2)]
                fuse = {}
                fcnt = [0] * NT
                dve_ks = (5,) + DVE_KS if j == 0 else DVE_KS
                for mt2 in range(MT2):
                    lp = psL.tile([128, 2, NBLK], F32, tag="L", name="L")
                    for sub in range(2):
                        mt = 2 * mt2 + sub
                        cc = (mt % 8) * 128
                        nc.tensor.matmul(
                            lp[:, sub, :],
                            x2c[mt // 8][:, :, cc:cc + 128],
                            q8[:, :, ns], start=True, stop=True, perf_mode=DR)
                    for item in avail_at.pop(S, ()):
                        fq.append(item)
                    pump(CAP)
                    emit_exp(e8, mt2, lp, dve_ks)
                    lag = 3 if mt2 in dve_ks else 2
                    avail_at.setdefault(S + lag, []).extend(
                        (e8, nt, mt2, fuse, fcnt, j) for nt in range(NT))
                    next(feed, None)
                    S += 1
        with nc.named_scope("tail"):
            while fq or avail_at:
                for item in avail_at.pop(S, ()):
                    fq.append(item)
                pump(CAP)
                S += 1
    nc.compile()
    return nc


_NC = None


def _get_nc():
    global _NC
    if _NC is None:
        _NC = build()
    return _NC


def kernel(**inputs):
    x1 = np.asarray(inputs["x1"], np.float32).reshape(B, C, N)
    x2 = np.asarray(inputs["x2"], np.float32).reshape(B, C, N)
    q_w = np.asarray(inputs["q_w"], np.float32)
    k_w = np.asarray(inputs["k_w"], np.float32)
    v_w = np.asarray(inputs["v_w"], np.float32)
    p_w = np.asarray(inputs["proj_w"], np.float32)
    q_b = np.asarray(inputs["q_b"], np.float32)
    v_b = np.asarray(inputs["v_b"], np.float32)
    p_b = np.asarray(inputs["proj_b"], np.float32)
    gamma = np.asarray(inputs["bn_gamma"], np.float32)
    beta = np.asarray(inputs["bn_beta"], np.float32)
    mean = np.asarray(inputs["bn_mean"], np.float32)
    var = np.asarray(inputs["bn_var"], np.float32)
    gate_w = np.asarray(inputs["gate_w"], np.float32)
    gate_b = np.asarray(inputs["gate_b"], np.float32)

    wqk = q_w.T @ k_w                      # [C,C]
    A = gamma[:, None] / np.sqrt(var + EPS)[:, None] * (p_w @ v_w)  # G*(pw vw)
    G = gamma / np.sqrt(var + EPS)
    Bc = (beta + (p_b + p_w @ v_b - mean) * G).astype(np.float32)
    qpb = (k_w.T @ q_b).astype(np.float32)
    bct = np.ascontiguousarray(np.broadcast_to(Bc, (128, C)))

    in_maps = []
    for b in range(B):
        Qp = (wqk.T @ x1[b] + qpb[:, None]).astype(NP8)      # [C, N]
        Y8 = (A @ x2[b]).astype(NP8)                          # [C, N]
        glog = gate_w[0, :C] @ x1[b] + gate_w[0, C:] @ x2[b] + gate_b[0]
        gate = (1.0 / (1.0 + np.exp(-glog))).astype(np.float32)  # [N]
        x28 = x2[b].astype(NP8)
        # x2dr/q8 layouts: [p, h, m] = arr[h*128+p, m]
        x2dr = np.ascontiguousarray(
            x28.reshape(2, 128, N).transpose(1, 0, 2).reshape(128, 2 * N))
        yt = np.zeros((128, MT, YW), NP8)
        yt[:, :, :C] = np.ascontiguousarray(
            Y8.reshape(C, MT, 128).transpose(2, 1, 0))
        yt[:, :, C] = np.float32(1.0)
        yt = np.ascontiguousarray(yt.reshape(128, MT * YW))
        for half in range(2):
            hq = slice(half * NH, (half + 1) * NH)
            q8 = np.ascontiguousarray(
                Qp[:, hq].reshape(2, 128, NH).transpose(1, 0, 2)
                .reshape(128, 2 * NH))
            x1t = np.ascontiguousarray(
                x1[b][:, hq].T.astype(ml_dtypes.bfloat16))    # [NH, C]
            gc = np.ascontiguousarray(
                gate[hq].reshape(NBLOCKS * NT, 128).T.astype(np.float32))
            in_maps.append({
                "q8": q8, "x2dr": x2dr, "yt": yt, "x1t": x1t,
                "gatec": gc, "bct": bct,
            })

    nc = _get_nc()
    res = run_bass_kernel_spmd(nc, in_maps, core_ids=list(range(NCORES)))
    out = np.empty((B, C, N), np.float32)
    for core in range(NCORES):
        b, half = divmod(core, 2)
        out[b, :, half * NH:(half + 1) * NH] = \
            res.results[core]["out"].astype(np.float32).T
    return out.reshape(B, C, H, W)
